# revision 1
# baseline (speedup 1.0000x reference)
"""Multi-head causal attention (b=4, n=2048, d=1024, h=16) on 8 trn2 cores.

Sharding: data-parallel over batch (4) x tensor-parallel over heads (2 groups
of 8 heads).  Core c handles batch c//2, heads 8*(c%2)..8*(c%2)+8.

Per-core dataflow (all matmuls in float32r: full PE rate, ~2e-4 rel err):
  xs[n] [128,8,512]   = x[b].T strip           (streamed per 512-col strip)
  w{q,k,v}T [1024,512] = W.T[:, group]         (host-prepared)
  KT [512,2048]       = wkT.T @ xT             (K transposed: [head*dim, n])
  Vp [2048, 8*65]     = xT.T @ wvT (+ ones col)
  QTs [512,512]       = wqT.T @ xs[n]          (per strip)
  per (head-pair, strip):
    S^T blocks [128 nk, 512 nq] = KT_h_blk.T @ QTs_h  (K=64 contraction;
      the pair's QK matmuls alternate PE row groups 0-63/64-127 so they
      stream concurrently; 2 blocks per PSUM chunk -> one exp(S/8) each)
    causal: 0/1 triangular multiply on the diagonal 128x128 sub-block (DVE,
      SBUF); QK and PV skip fully-masked column ranges
    [O^T; sums] accumulated in PSUM = [V|1]_blk.T @ P^T_blk over k blocks
    normalize on device: O^T * bcast(1/sums); the partition-broadcast runs
    on GPSIMD (raw sbuf tensors), then -> DRAM (Pool SWDGE)
  host gather: out[b, :, group] = outT.T

Each strip's projection groups (KT/Vp/QTs of that strip) are woven into its
own attention emission - one group per exp chunk - so the PE stays fed while
ACT (the attention bottleneck) drains the exp queue; per pair t only
[KT m=t, QT m=t] must precede it, and the new Vp blocks are woven at
2/chunk during pair 0 ahead of the diagonal PV matmuls that read them.
"""

import numpy as np

import concourse.bacc as bacc
import concourse.mybir as mybir
import concourse.tile as tile
from concourse import bass_utils
from concourse.bass_interp import get_hw_module

N_CORES = 8
B, N, D = 4, 2048, 1024
HEADS = 16
HPC = 8            # heads per core
HD = 64            # head dim
GD = HPC * HD      # 512 weight columns per core
KC = D // 128      # 8 contraction chunks of in_dim
NB = N // 128      # 16 key blocks
NSTRIP = N // 512  # 4 query strips
CH = 2             # S^T key-blocks per PSUM chunk / exp call
PSS_BUFS = 2       # PSUM: 2*2 banks S chunks + 1 proj + 2 PV accum = 7 of 8

f32 = mybir.dt.float32
f32r = mybir.dt.float32r
EXP = mybir.ActivationFunctionType.Exp


def build_program():
    nc = bacc.Bacc("TRN2", target_bir_lowering=False, debug=False,
                   num_devices=N_CORES)
    xT = nc.dram_tensor("xT", [D, N], f32r, kind="ExternalInput").ap()
    wqT = nc.dram_tensor("wqT", [D, GD], f32r, kind="ExternalInput").ap()
    wkT = nc.dram_tensor("wkT", [D, GD], f32r, kind="ExternalInput").ap()
    wvT = nc.dram_tensor("wvT", [D, GD], f32r, kind="ExternalInput").ap()
    tri01 = nc.dram_tensor("tri01", [128, 128], f32, kind="ExternalInput").ap()
    outT = nc.dram_tensor("outT", [GD, N], f32, kind="ExternalOutput").ap()
    # raw (non-pool) sbuf tensors: partition_broadcast needs concrete APs
    rec_raw = [nc.alloc_sbuf_tensor(f"rec_raw{i}", [1, 512], f32).ap()
               for i in range(2)]
    rb_raw = [nc.alloc_sbuf_tensor(f"rb_raw{i}", [64, 512], f32).ap()
              for i in range(2)]

    with tile.TileContext(nc) as tc:
        with (
            tc.tile_pool(name="xs", bufs=2) as xs_pool,
            tc.tile_pool(name="w", bufs=1) as w_pool,
            tc.tile_pool(name="big", bufs=1) as big_pool,
            tc.tile_pool(name="qt", bufs=3) as qt_pool,
            tc.tile_pool(name="pt", bufs=4) as pt_pool,
            tc.tile_pool(name="ot", bufs=3) as ot_pool,
            tc.tile_pool(name="small", bufs=1) as small_pool,
            tc.tile_pool(name="ps_s", bufs=PSS_BUFS, space="PSUM") as ps_s,
            tc.tile_pool(name="ps_proj", bufs=1, space="PSUM") as ps_proj,
            tc.tile_pool(name="ps_o", bufs=3, space="PSUM") as ps_o,
        ):
            wq_t = w_pool.tile([128, KC, GD], f32r, tag="wq")
            wk_t = w_pool.tile([128, KC, GD], f32r, tag="wk")
            wv_t = w_pool.tile([128, KC, GD], f32r, tag="wv")
            # interleave wk and strip-0 x chunks: the first KT projection
            # group consumes them in k order, so it starts after ~2 DMAs
            # instead of waiting behind all the weight loads
            xs0 = xs_pool.tile([128, KC, 512], f32r, tag="xs", name="xs0")
            for k in range(KC):
                nc.sync.dma_start(wk_t[:, k, :],
                                  wkT[k * 128:(k + 1) * 128, :])
                nc.sync.dma_start(xs0[:, k, :],
                                  xT[k * 128:(k + 1) * 128, 0:512])
            for wt, wd in ((wq_t, wqT), (wv_t, wvT)):
                for k in range(KC):
                    nc.sync.dma_start(wt[:, k, :],
                                      wd[k * 128:(k + 1) * 128, :])
            tri = small_pool.tile([128, 128], f32, tag="tri")
            nc.sync.dma_start(tri[:], tri01[:])
            # warm the ACT exp table while input DMAs stream
            warmup = small_pool.tile([1, 1], f32, tag="warmup")
            nc.vector.memset(warmup[:], 0.0)
            nc.scalar.activation(warmup[:], warmup[:], EXP)

            kt = big_pool.tile([128, 4, N], f32r, tag="kt")
            vp = big_pool.tile([128, NB, HPC, HD + 1], f32r, tag="vp")
            # ones column: init whole tile (contiguous memset); V copies
            # overwrite the value columns
            nc.vector.memset(
                vp[:].rearrange("p a b c -> p (a b c)").bitcast(f32), 1.0)

            def load_strip(n):
                xs = xs_pool.tile([128, KC, 512], f32r, tag="xs")
                for k in range(KC):
                    nc.sync.dma_start(
                        xs[:, k, :],
                        xT[k * 128:(k + 1) * 128, n * 512:(n + 1) * 512])
                return xs

            def proj_group(lhs_fn, rhs_fn, copy_out_fn):
                ps = ps_proj.tile([128, 512], f32, tag="psp", name="psp")
                for k in range(KC):
                    nc.tensor.matmul(ps[:], lhs_fn(k), rhs_fn(k),
                                     start=(k == 0), stop=(k == KC - 1))
                copy_out_fn(ps)

            def emit_strip_projections(n, xs, qts):
                """List of closures, one PE-sized projection group each."""
                groups = []
                for m in range(4):      # K^T rows m*128.. for strip n
                    groups.append(lambda m=m: proj_group(
                        lambda k, m=m: wk_t[:, k, m * 128:(m + 1) * 128],
                        lambda k: xs[:, k, :],
                        lambda ps, m=m: nc.vector.tensor_copy(
                            kt[:, m, n * 512:(n + 1) * 512], ps[:]),
                    ))
                for i in range(4):      # V blocks 4n+i
                    mt = 4 * n + i
                    groups.append(lambda mt=mt, i=i: proj_group(
                        lambda k, i=i: xs[:, k, i * 128:(i + 1) * 128],
                        lambda k: wv_t[:, k, :],
                        lambda ps, mt=mt: nc.vector.tensor_copy(
                            vp[:, mt, :, 0:HD],
                            ps[:].rearrange("p (h d) -> p h d", h=HPC)),
                    ))
                for m in range(4):      # Q^T strip n rows m*128..
                    groups.append(lambda m=m: proj_group(
                        lambda k, m=m: wq_t[:, k, m * 128:(m + 1) * 128],
                        lambda k: xs[:, k, :],
                        lambda ps, m=m: nc.vector.tensor_copy(
                            qts[:, m, :], ps[:]),
                    ))
                return groups

            def emit_pv(po, ptc, jj, h, qs, nblocks):
                for idx, j in enumerate(jj):
                    r = j - 4 * qs
                    nstart = 128 * r if r > 0 else 0
                    nc.tensor.matmul(
                        po[:, nstart:512],
                        vp[:, j, h, :],
                        ptc[:, idx, nstart:512],
                        start=(j == 0), stop=(j == nblocks - 1),
                    )

            def attention_pair(h0, qs, qts, weave_fn=None):
                """Heads (h0, h0+1): h0 on PE rows 0-63, h0+1 on rows 64-127.
                QK matmuls interleave the two heads so adjacent MMs hit
                disjoint row groups and stream concurrently."""
                nblocks = 4 * qs + 4
                heads = (h0, h0 + 1)
                m = h0 // 2
                po = {h: ps_o.tile([HD + 1, 512], f32, tag="po",
                                   name=f"po_h{h}")
                      for h in heads}
                pending = {h: None for h in heads}
                for c0 in range(0, nblocks, CH):
                    jj = list(range(c0, min(c0 + CH, nblocks)))
                    w = len(jj)
                    pss = {h: ps_s.tile([128, CH, 512], f32, tag="pss",
                                        name=f"pss_h{h}")
                           for h in heads}
                    for idx, j in enumerate(jj):
                        r = j - 4 * qs
                        nstart = 128 * r if 0 < r < 3 else 0
                        for h in heads:
                            p0 = (h % 2) * 64
                            nc.tensor.matmul(
                                pss[h][:, idx, nstart:512],
                                kt[p0:p0 + 64, m, j * 128:(j + 1) * 128],
                                qts[p0:p0 + 64, m, nstart:512],
                                start=True, stop=True,
                            )
                    ptc = {}
                    for h in heads:
                        ptc[h] = pt_pool.tile([128, CH, 512], f32r,
                                              tag="ptc", name=f"ptc_h{h}")
                        nc.scalar.activation(ptc[h][:, 0:w, :],
                                             pss[h][:, 0:w, :],
                                             EXP, scale=0.125)
                        # causal 0/1 mask on diagonal sub-blocks (SBUF)
                        for idx, j in enumerate(jj):
                            r = j - 4 * qs
                            if r >= 0:
                                nc.vector.tensor_mul(
                                    ptc[h][:, idx, r * 128:(r + 1) * 128],
                                    ptc[h][:, idx, r * 128:(r + 1) * 128],
                                    tri[:],
                                )
                    if weave_fn is not None:
                        weave_fn()
                    for h in heads:
                        if pending[h] is not None:
                            emit_pv(po[h], pending[h][0], pending[h][1],
                                    h, qs, nblocks)
                        pending[h] = (ptc[h], jj)
                for h in heads:
                    emit_pv(po[h], pending[h][0], pending[h][1], h, qs,
                            nblocks)
                    # normalize on device: otile = O^T * bcast(1/sums);
                    # the partition-broadcast runs on GPSIMD (raw sbuf
                    # tensors: the op needs concrete, non-pool APs)
                    i = h % 2
                    nc.vector.reciprocal(rec_raw[i][:],
                                         po[h][HD:HD + 1, :])
                    nc.gpsimd.partition_broadcast(rb_raw[i][:],
                                                  rec_raw[i][:])
                    otile = ot_pool.tile([64, 512], f32, tag="otile",
                                         name=f"otile{h}", bufs=3)
                    nc.vector.tensor_mul(otile[:], po[h][0:HD, :],
                                         rb_raw[i][:])
                    nc.gpsimd.dma_start(
                        outT[h * HD:(h + 1) * HD,
                             qs * 512:(qs + 1) * 512],
                        otile[:],
                    )

            # ---- main emission (self-hosted strips): each strip's
            # projection groups are woven into its OWN attention gaps.
            # Per pair t, only [KT m=t, QT m=t] must precede it; VP groups
            # are woven at 2/chunk during pair 0, ahead of the diagonal
            # PV matmuls that consume them.
            xs = xs0
            for qs in range(NSTRIP):
                qts = qt_pool.tile([128, 4, 512], f32r, tag="qts",
                                   name=f"qts{qs}")
                g = emit_strip_projections(qs, xs, qts)
                # g order: [KT m0..3, VP x4, QT m0..3]
                queue = ([("kq", 0, g[0]), ("kq", 0, g[8])] +
                         [("vp", None, g[4 + i]) for i in range(4)] +
                         [("kq", mm, fn) for mm in (1, 2, 3)
                          for fn in (g[mm], g[8 + mm])])
                # pair-0 requirements upfront
                queue.pop(0)[2]()
                queue.pop(0)[2]()
                if qs + 1 < NSTRIP:
                    xs = load_strip(qs + 1)

                def weave_fn():
                    n = 2 if (queue and queue[0][0] == "vp") else 1
                    for _ in range(n):
                        if queue:
                            queue.pop(0)[2]()

                for t in range(HPC // 2):
                    while queue and any(k == "kq" and mm <= t
                                        for k, mm, _ in queue):
                        queue.pop(0)[2]()
                    attention_pair(2 * t, qs, qts, weave_fn)
                for item in queue:
                    item[2]()

    nc.compile()
    nc.m = get_hw_module(nc.m)
    return nc


_PROGRAM = None


def _program():
    global _PROGRAM
    if _PROGRAM is None:
        _PROGRAM = build_program()
    return _PROGRAM


def make_in_maps(x, Wq, Wk, Wv):
    kk, qq = np.meshgrid(np.arange(128), np.arange(128), indexing="ij")
    tri = (qq >= kk).astype(np.float32)
    in_maps = []
    for c in range(N_CORES):
        b, g = c // 2, c % 2
        sl = slice(g * GD, (g + 1) * GD)
        in_maps.append({
            "xT": np.ascontiguousarray(np.asarray(x)[b].T),
            "wqT": np.ascontiguousarray(np.asarray(Wq).T[:, sl]),
            "wkT": np.ascontiguousarray(np.asarray(Wk).T[:, sl]),
            "wvT": np.ascontiguousarray(np.asarray(Wv).T[:, sl]),
            "tri01": tri,
        })
    return in_maps


def gather(results):
    out = np.empty((B, N, D), np.float32)
    for c in range(N_CORES):
        b, g = c // 2, c % 2
        out[b, :, g * GD:(g + 1) * GD] = results[c]["outT"].T
    return out


def kernel(x, Wq, Wk, Wv):
    nc = _program()
    in_maps = make_in_maps(x, Wq, Wk, Wv)
    res = bass_utils.run_bass_kernel_spmd(nc, in_maps,
                                          core_ids=list(range(N_CORES)))
    return gather(res.results)



# revision 17
# speedup vs baseline: 1.1563x; 1.1563x over previous
"""Multi-head causal attention (b=4, n=2048, d=1024, h=16) on 8 trn2 cores.

Sharding: data-parallel over batch (4) x tensor-parallel over heads (2 groups
of 8 heads).  Core c handles batch c//2, heads 8*(c%2)..8*(c%2)+8.

fp8 (e4m3) DoubleRow matmuls for projections and QK at 0.5 cyc/row:
  - Host prep: W' = 32*W split into (W8, Wr8) e4m3 pair; x split into
    (x8, xr8).  DoubleRow layout [128, 4 k-tiles, 2 halves, cols].
  - Projections: 3-term compensation W8*x8 + W8*xr8 + Wr8*x8 (12 DR matmuls
    per [128,512] PSUM tile vs 8 f32r) -> ~0.4% proj error.
  - QK: per key block one DR matmul with planes (K8, Kr8) x (Q8, Q8):
    = (K8+Kr8)^T Q8 -- K compensated for free, Q single e4m3 cast.
  - PV stays f32r (P quantization would break the 2e-2 gate on early rows).
  - exp scale absorbs the 32*32 weight scaling (0.125/1024); output is
    unnormalized [O^T; sums] -> DRAM; host divides by sums and 32.
Measured end-to-end rel err ~1.1e-2 (gate 2e-2) via the numpy quant model.

Attention emission mirrors the woven baseline: S^T blocks [128 keys, 512 q]
per chunk of 2, exp on ACT (the bottleneck engine) with diagonal chunks
restricted to cols 256:512, causal 0/1 multiplies on DVE, per-strip
projection groups woven into the exp gaps so the PE never drains.
"""

import numpy as np

import concourse.bacc as bacc
import concourse.mybir as mybir
import concourse.tile as tile
from concourse import bass_utils
from concourse.bass_interp import get_hw_module

N_CORES = 8
B, N, D = 4, 2048, 1024
HEADS = 16
HPC = 8            # heads per core
HD = 64            # head dim
GD = HPC * HD      # 512 weight columns per core
KC4 = 4            # 256-deep DoubleRow contraction tiles over in_dim
NB = N // 128      # 16 key blocks
NSTRIP = N // 512  # 4 query strips
CH = 2             # S^T key-blocks per PSUM chunk / exp call
SCALE = 32.0       # host-applied weight scale (fp8 residual range)
EXP_SCALE = 0.125 / (SCALE * SCALE)

f32 = mybir.dt.float32
f32r = mybir.dt.float32r
f8 = mybir.dt.float8e4
EXP = mybir.ActivationFunctionType.Exp
DR = mybir.MatmulPerfMode.DoubleRow


def build_program():
    nc = bacc.Bacc("TRN2", target_bir_lowering=False, debug=False,
                   num_devices=N_CORES)
    # strip-major x layout: [:, s] is contiguous 4KB/partition -> one
    # descriptor per partition per strip load (SP.SEQ/HWDGE are serial
    # resources at ~625ns+desc cost per DMA; strided per-k slices were 8x
    # the descriptor count and serialized startup)
    x8d = nc.dram_tensor("x8", [128, NSTRIP, KC4, 2, 512], f8,
                         kind="ExternalInput").ap()
    xr8d = nc.dram_tensor("xr8", [128, NSTRIP, KC4, 2, 512], f8,
                          kind="ExternalInput").ap()
    wd = {}
    for nm in ("q", "k", "v"):
        wd[nm + "8"] = nc.dram_tensor(f"w{nm}8", [128, KC4, 2, GD], f8,
                                      kind="ExternalInput").ap()
        wd[nm + "r8"] = nc.dram_tensor(f"w{nm}r8", [128, KC4, 2, GD], f8,
                                       kind="ExternalInput").ap()
    trid = nc.dram_tensor("tri01", [128, 128], f32, kind="ExternalInput").ap()
    ztrid = nc.dram_tensor("ztri01", [128, 256], f32,
                           kind="ExternalInput").ap()
    out65d = nc.dram_tensor("out65", [HD + 1, HPC, N], f32,
                            kind="ExternalOutput").ap()

    with tile.TileContext(nc) as tc:
        with (
            tc.tile_pool(name="xs", bufs=2) as xs_pool,
            tc.tile_pool(name="w", bufs=1) as w_pool,
            tc.tile_pool(name="big", bufs=1) as big_pool,
            tc.tile_pool(name="qt", bufs=3) as qt_pool,
            tc.tile_pool(name="pt", bufs=4) as pt_pool,
            tc.tile_pool(name="o", bufs=3) as o_pool,
            tc.tile_pool(name="small", bufs=1) as small_pool,
            tc.tile_pool(name="ps_s", bufs=2, space="PSUM") as ps_s,
            tc.tile_pool(name="ps_proj", bufs=1, space="PSUM") as ps_proj,
            tc.tile_pool(name="ps_o", bufs=3, space="PSUM") as ps_o,
        ):
            wk8_t = w_pool.tile([128, KC4, 2, GD], f8, tag="wk8")
            wkr8_t = w_pool.tile([128, KC4, 2, GD], f8, tag="wkr8")
            wq8_t = w_pool.tile([128, KC4, 2, GD], f8, tag="wq8")
            wqr8_t = w_pool.tile([128, KC4, 2, GD], f8, tag="wqr8")
            wv8_t = w_pool.tile([128, KC4, 2, GD], f8, tag="wv8")
            wvr8_t = w_pool.tile([128, KC4, 2, GD], f8, tag="wvr8")
            # whole-tile DMAs: 4KB/partition contiguous, 128 descriptors
            x8s0 = xs_pool.tile([128, KC4, 2, 512], f8, tag="x8s",
                                name="x8s0")
            xr8s0 = xs_pool.tile([128, KC4, 2, 512], f8, tag="xr8s",
                                 name="xr8s0")
            # startup-critical order: the first projection groups are QT0
            # (terms (wq8,x8)k*, (wq8,xr8)k*, (wqr8,x8)k*) then KT0; feed
            # the PE with k01-half loads so matmuls start ~2.5us in
            def half(dst, src, h):
                nc.sync.dma_start(dst[:, 2 * h:2 * h + 2, :, :],
                                  src[:, 2 * h:2 * h + 2, :, :])
            for h in range(2):
                half(wq8_t, wd["q8"], h)
                nc.sync.dma_start(x8s0[:, 2 * h:2 * h + 2, :, :],
                                  x8d[:, 0, 2 * h:2 * h + 2, :, :])
                nc.sync.dma_start(xr8s0[:, 2 * h:2 * h + 2, :, :],
                                  xr8d[:, 0, 2 * h:2 * h + 2, :, :])
                half(wqr8_t, wd["qr8"], h)
            for h in range(2):
                half(wk8_t, wd["k8"], h)
                half(wkr8_t, wd["kr8"], h)
            nc.sync.dma_start(wv8_t[:], wd["v8"][:])
            nc.sync.dma_start(wvr8_t[:], wd["vr8"][:])
            tri = small_pool.tile([128, 128], f32, tag="tri")
            ztri = small_pool.tile([128, 256], f32, tag="ztri")
            nc.sync.dma_start(tri[:], trid[:])
            nc.sync.dma_start(ztri[:], ztrid[:])
            # warm the ACT exp table while input DMAs stream
            warmup = small_pool.tile([1, 1], f32, tag="warmup")
            nc.vector.memset(warmup[:], 0.0)
            nc.scalar.activation(warmup[:], warmup[:], EXP)
            # warm the PE p-state during the DMA dead time: ~3us of dummy
            # DR matmuls on zeroed tiles so the first projection chain runs
            # at full clock (2.4GHz) instead of the 1.2GHz mid p-state
            wml = small_pool.tile([128, 2, 128], f8, tag="wml")
            wmr = small_pool.tile([128, 2, 512], f8, tag="wmr")
            nc.vector.memset(
                wml[:].rearrange("p a b -> p (a b)").bitcast(f32), 0.0)
            nc.vector.memset(
                wmr[:].rearrange("p a b -> p (a b)").bitcast(f32), 0.0)
            wps = ps_proj.tile([128, 512], f32, tag="psp", name="psp")
            for _ in range(14):
                nc.tensor.matmul(wps[:], wml[:], wmr[:], start=True,
                                 stop=True, perf_mode=DR,
                                 skip_group_check=True)

            k8r = big_pool.tile([128, 4, 2, N], f8, tag="k8r")
            vp = big_pool.tile([128, NB, HPC, HD + 1], f32r, tag="vp")
            # ones column only (128 strided elems/partition, on the idle
            # Pool engine): a full-tile memset is 8.7us of DVE time that
            # blocks the startup projection copies behind it
            nc.gpsimd.memset(vp[:, :, :, HD:HD + 1].bitcast(f32), 1.0)

            def load_strip(n):
                a = xs_pool.tile([128, KC4, 2, 512], f8, tag="x8s",
                                 name=f"x8s{n}")
                b = xs_pool.tile([128, KC4, 2, 512], f8, tag="xr8s",
                                 name=f"xr8s{n}")
                nc.sync.dma_start(a[:], x8d[:, n])
                nc.sync.dma_start(b[:], xr8d[:, n])
                return a, b

            def proj_group(terms, copy_out):
                ps = ps_proj.tile([128, 512], f32, tag="psp", name="psp")
                nt = len(terms)
                for i, (lhs, rhs) in enumerate(terms):
                    nc.tensor.matmul(ps[:], lhs, rhs, start=(i == 0),
                                     stop=(i == nt - 1), perf_mode=DR)
                copy_out(ps)

            def emit_strip_projections(n, x8s, xr8s, qts):
                """List of closures, one PE-sized projection group each."""
                nsl = slice(n * 512, (n + 1) * 512)
                groups = []

                def kt_group(m):
                    terms = []
                    for kh in range(2):   # k-half-major: start on half loads
                        for wa, xa in ((wk8_t, x8s), (wk8_t, xr8s),
                                       (wkr8_t, x8s)):
                            for k in (2 * kh, 2 * kh + 1):
                                terms.append(
                                    (wa[:, k, :, m * 128:(m + 1) * 128],
                                     xa[:, k, :, :]))

                    def co(ps):
                        nc.vector.tensor_copy(k8r[:, m, 0, nsl], ps[:])
                        nc.vector.tensor_sub(k8r[:, m, 1, nsl], ps[:],
                                             k8r[:, m, 0, nsl])
                    proj_group(terms, co)

                def vp_group(i):
                    mt = 4 * n + i
                    terms = []
                    for xa, wa in ((x8s, wv8_t), (xr8s, wv8_t),
                                   (x8s, wvr8_t)):
                        for k in range(KC4):
                            terms.append((xa[:, k, :, i * 128:(i + 1) * 128],
                                          wa[:, k, :, :]))
                    # (no half ordering: wv loads last anyway)

                    def co(ps):
                        nc.vector.tensor_copy(
                            vp[:, mt, :, 0:HD],
                            ps[:].rearrange("p (h d) -> p h d", h=HPC))
                    proj_group(terms, co)

                def qt_group(m):
                    terms = []
                    for kh in range(2):
                        for wa, xa in ((wq8_t, x8s), (wq8_t, xr8s),
                                       (wqr8_t, x8s)):
                            for k in (2 * kh, 2 * kh + 1):
                                terms.append(
                                    (wa[:, k, :, m * 128:(m + 1) * 128],
                                     xa[:, k, :, :]))

                    def co(ps):
                        nc.vector.tensor_copy(qts[:, m, 0, :], ps[:])
                        nc.vector.tensor_copy(qts[:, m, 1, :], ps[:])
                    proj_group(terms, co)

                for m in range(4):
                    groups.append(lambda m=m: kt_group(m))
                for i in range(4):
                    groups.append(lambda i=i: vp_group(i))
                for m in range(4):
                    groups.append(lambda m=m: qt_group(m))
                return groups

            def emit_pv(po_h, ptc_h, jj, h, qs, nblocks):
                for idx, j in enumerate(jj):
                    r = j - 4 * qs
                    cs = 0 if r < 1 else (128 if r == 1 else 256)
                    nc.tensor.matmul(
                        po_h[:, cs:512],
                        vp[:, j, h, :],
                        ptc_h[:, idx, cs:512],
                        start=(j == 0), stop=(j == nblocks - 1),
                    )

            class PairUnit:
                """Heads (h0, h0+1): h0 on PE rows 0-63, h0+1 on rows
                64-127.  One fp8 DR matmul per (head, key block): planes
                (K8, Kr8) against duplicated Q8.  emit_chunk(0) of the next
                pair is emitted BEFORE this pair's emit_tail() so the
                final deferred PV + output copy hide under the next exp."""

                def __init__(self, h0, qs, qts, weave_fn):
                    self.h0, self.qs, self.qts = h0, qs, qts
                    self.weave_fn = weave_fn
                    self.nblocks = 4 * (qs + 1)
                    self.nchunks = self.nblocks // CH
                    self.heads = (h0, h0 + 1)
                    self.m = h0 // 2
                    self.po = {h: ps_o.tile([HD + 1, 512], f32, tag="po",
                                            name=f"po_h{h}")
                               for h in self.heads}
                    self.pending = {h: None for h in self.heads}

                def emit_chunk(self, ci):
                    qs, m, heads, qts = self.qs, self.m, self.heads, self.qts
                    jj = [CH * ci, CH * ci + 1]
                    last = (ci == self.nchunks - 1)  # diagonal (r2, r3)
                    c0 = 256 if last else 0
                    pss = {h: ps_s.tile([128, CH, 512], f32, tag="pss",
                                        name=f"pss_h{h}")
                           for h in heads}
                    for idx, j in enumerate(jj):
                        for h in heads:
                            p0 = (h % 2) * 64
                            nc.tensor.matmul(
                                pss[h][:, idx, c0:512],
                                k8r[p0:p0 + 64, m, :,
                                    j * 128:(j + 1) * 128],
                                qts[p0:p0 + 64, m, :, c0:512],
                                start=True, stop=True, perf_mode=DR,
                            )
                    ptc = {}
                    for h in heads:
                        ptc[h] = pt_pool.tile([128, CH, 512], f32r,
                                              tag="ptc", name=f"ptc_h{h}")
                        nc.scalar.activation(ptc[h][:, 0:CH, c0:512],
                                             pss[h][:, 0:CH, c0:512],
                                             EXP, scale=EXP_SCALE)
                        # causal 0/1 masks on diagonal sub-blocks: on the
                        # (otherwise idle) GPSIMD engine so they never queue
                        # behind DVE's projection copies
                        if last:                    # blocks (r2, r3)
                            nc.gpsimd.tensor_mul(
                                ptc[h][:, 0, 256:384],
                                ptc[h][:, 0, 256:384], tri[:])
                            nc.gpsimd.tensor_mul(
                                ptc[h][:, 1, 256:512],
                                ptc[h][:, 1, 256:512], ztri[:])
                        elif ci == self.nchunks - 2:  # blocks (r0, r1)
                            nc.gpsimd.tensor_mul(
                                ptc[h][:, 0, 0:128],
                                ptc[h][:, 0, 0:128], tri[:])
                            nc.gpsimd.tensor_mul(
                                ptc[h][:, 1, 128:256],
                                ptc[h][:, 1, 128:256], tri[:])
                    self.weave_fn()
                    for h in heads:
                        if self.pending[h] is not None:
                            emit_pv(self.po[h], self.pending[h][0],
                                    self.pending[h][1], h, qs, self.nblocks)
                        self.pending[h] = (ptc[h], jj)

                def emit_tail(self):
                    qs = self.qs
                    for h in self.heads:
                        emit_pv(self.po[h], self.pending[h][0],
                                self.pending[h][1], h, qs, self.nblocks)
                        # unnormalized [O^T; sums] -> SBUF -> DRAM; the
                        # host divides by sums and the weight scale
                        o65 = o_pool.tile([HD + 1, 512], f32, tag="o65",
                                          name=f"o65_{h}")
                        nc.vector.tensor_copy(o65[:], self.po[h][:])
                        nc.sync.dma_start(
                            out65d[:, h, qs * 512:(qs + 1) * 512], o65[:])

            # ---- main emission: a single GLOBAL projection queue.  Strip
            # s+1's groups are appended while strip s's attention runs, so
            # the PE-heavy early strips drain projection work for the
            # ACT-heavy late strips (by strip 3 the queue is empty and ACT
            # runs saturated).  Need-order inside a strip:
            #   KT0 QT0 | VP0-3 | KT1 QT1 KT2 QT2 KT3 QT3
            # Safety pops: (strip<qs) or (kq, m<=t) before pair t; remaining
            # (qs, vp) before the last chunk of pair 0 (ahead of the
            # diagonal PV that reads vp).
            queue = []   # entries: (strip, kind, m, fn)
            qts_tiles = {}
            xs_tiles = {0: (x8s0, xr8s0)}

            def append_strip(s):
                qts_tiles[s] = qt_pool.tile([128, 4, 2, 512], f8, tag="qts",
                                            name=f"qts{s}")
                g = emit_strip_projections(s, xs_tiles[s][0], xs_tiles[s][1],
                                           qts_tiles[s])
                # g order: [KT m0..3, VP x4, QT m0..3]
                queue.append((s, "kq", 0, g[0]))
                queue.append((s, "kq", 0, g[8]))
                for i in range(4):
                    queue.append((s, "vp", i, g[4 + i]))
                for mm in (1, 2, 3):
                    queue.append((s, "kq", mm, g[mm]))
                    queue.append((s, "kq", mm, g[8 + mm]))

            def force_pop(pred):
                kept = []
                for e in queue:
                    if pred(e):
                        e[3]()
                    else:
                        kept.append(e)
                queue[:] = kept

            append_strip(0)
            # pair-0-of-strip-0 requirements upfront (QT0 first: its
            # weight halves lead the DMA order)
            queue.pop(1)[3]()
            queue.pop(0)[3]()
            gchunk = [0]     # global chunk counter (weave pacing)
            prev_unit = [None]

            def weave_fn():
                # strips 0-1: drain eagerly (PE-deficit phase anyway);
                # strip 2: alternate chunks (matches its exp slack);
                # strip 3: queue should be empty
                qs = cur_qs[0]
                n_ = 2 if qs == 0 else (1 if qs == 1 or gchunk[0] % 2 == 0
                                        else 0)
                for _ in range(n_):
                    if queue:
                        queue.pop(0)[3]()

            cur_qs = [0]
            for qs in range(NSTRIP):
                cur_qs[0] = qs
                if qs + 1 < NSTRIP:
                    xs_tiles[qs + 1] = load_strip(qs + 1)
                    append_strip(qs + 1)
                for t in range(HPC // 2):
                    force_pop(lambda e, qs=qs, t=t: (
                        e[0] < qs or (e[0] == qs and e[1] == "kq"
                                      and e[2] <= t)))
                    u = PairUnit(2 * t, qs, qts_tiles[qs], weave_fn)
                    for ci in range(u.nchunks):
                        if t == 0 and ci == u.nchunks - 1:
                            force_pop(lambda e, qs=qs: (e[0] == qs
                                                        and e[1] == "vp"))
                        u.emit_chunk(ci)
                        gchunk[0] += 1
                        if ci == 0 and prev_unit[0] is not None:
                            prev_unit[0].emit_tail()
                            prev_unit[0] = None
                    prev_unit[0] = u
            prev_unit[0].emit_tail()
            for e in queue:
                e[3]()

    nc.compile()
    nc.m = get_hw_module(nc.m)
    return nc


_PROGRAM = None


def _program():
    global _PROGRAM
    if _PROGRAM is None:
        _PROGRAM = build_program()
    return _PROGRAM


def make_in_maps(x, Wq, Wk, Wv):
    import ml_dtypes
    E4 = ml_dtypes.float8_e4m3

    def q8(a):
        return a.astype(E4)

    def dr(a):  # [1024, cols] -> [128, 4 k-tiles, 2 halves, cols]
        return np.ascontiguousarray(
            a.reshape(4, 2, 128, a.shape[-1]).transpose(2, 0, 1, 3))

    kk, qq = np.meshgrid(np.arange(128), np.arange(128), indexing="ij")
    tri = (qq >= kk).astype(np.float32)
    ztri = np.concatenate([np.zeros((128, 128), np.float32), tri], axis=1)
    def strip_major(a):  # [128, 4, 2, 2048] -> [128, NSTRIP, 4, 2, 512]
        return np.ascontiguousarray(
            a.reshape(128, KC4, 2, NSTRIP, 512).transpose(0, 3, 1, 2, 4))

    x = np.asarray(x)
    xcache = {}
    for b in range(B):
        xb = np.ascontiguousarray(x[b].T).astype(np.float32)
        x8 = q8(xb)
        xr8 = q8(xb - x8.astype(np.float32))
        xcache[b] = (strip_major(dr(x8)), strip_major(dr(xr8)))
    in_maps = []
    for c in range(N_CORES):
        b, g = c // 2, c % 2
        sl = slice(g * GD, (g + 1) * GD)
        m = {"x8": xcache[b][0], "xr8": xcache[b][1],
             "tri01": tri, "ztri01": ztri}
        for nm, W in (("q", Wq), ("k", Wk), ("v", Wv)):
            Ws = (SCALE * np.asarray(W).T[:, sl]).astype(np.float32)
            W8 = q8(Ws)
            Wr8 = q8(Ws - W8.astype(np.float32))
            m[f"w{nm}8"] = dr(W8)
            m[f"w{nm}r8"] = dr(Wr8)
        in_maps.append(m)
    return in_maps


def gather(results):
    out = np.empty((B, N, D), np.float32)
    for c in range(N_CORES):
        b, g = c // 2, c % 2
        o = np.asarray(results[c]["out65"], np.float32)  # [65, 8, 2048]
        o_norm = o[0:HD] / o[HD][None, :, :] / SCALE     # [64, 8, 2048]
        out[b, :, g * GD:(g + 1) * GD] = (
            o_norm.transpose(2, 1, 0).reshape(N, GD))
    return out


def kernel(x, Wq, Wk, Wv):
    nc = _program()
    in_maps = make_in_maps(x, Wq, Wk, Wv)
    res = bass_utils.run_bass_kernel_spmd(nc, in_maps,
                                          core_ids=list(range(N_CORES)))
    return gather(res.results)
